# revision 16
# baseline (speedup 1.0000x reference)
"""MoE batched-experts kernel for Trainium2 (8 NeuronCores, expert-parallel).

Contract: kernel(**inputs) takes the FULL unsharded inputs
  x:              [T, D]      float32   (T=16384, D=1024)
  expert_indices: [T]         int32     (values in [0, 8))
  gate_up_weight: [E, 2F, D]  float32   (E=8, F=2048)
  down_weight:    [E, D, F]   float32
and returns the FULL output [T, D] float32:
  per token t with expert e: h = silu(x @ gu[e,:F].T) * (x @ gu[e,F:].T);
  out = h @ dw[e].T

Strategy: expert-parallel. The host routes (sorts) tokens by expert, pads
each expert's group to a common capacity C (max count rounded up to 8), and
core e runs a dense FFN for expert e on its token group. All operands are
pre-transposed / pre-cast to fp16 on the host (same PE rate as bf16, 8x the
mantissa: rel-err ~6e-4 vs ~4.5e-3) so the device kernel is pure matmul +
silu*mul with no on-chip transposes:
  core e computes outT = w_d @ (silu(w_gT.T @ xT) * (w_uT.T @ xT))
with xT [D, C], producing outT [D, C] fp32; the host transposes back and
unpermutes.
"""

import numpy as np
import ml_dtypes

import concourse.bass as bass
import concourse.mybir as mybir
from concourse import bacc
from concourse.tile import TileContext
from concourse.bass import ts, ds
from concourse.bass_utils import run_bass_kernel_spmd
from contextlib import ExitStack

BF16 = ml_dtypes.bfloat16
F16 = np.float16  # operand dtype for matmuls: same PE rate as bf16, 8x mantissa

D = 1024      # d_model
F = 2048      # d_ff
F2 = 2 * F    # gate+up
E = 8         # experts == cores
KD = D // 128   # 8  k-tiles over d_model
KF = F // 128   # 16 k-tiles over d_ff
MD = D // 128   # 8  m-tiles over d_model (output)
NT = 512        # token chunk (one PSUM bank at fp32)

_nc_cache = {}


def build_nc(C, repeats=1, hw_loop=0):
    """Build the per-core dense-FFN Bass program for token capacity C.

    repeats>1 re-emits the whole compute body (unrolled); hw_loop>0 wraps the
    body in a hardware For_i loop. Both are timing aids: slope of time vs
    repetition count isolates true exec time from dispatch overhead."""
    nc = bacc.Bacc("TRN2", target_bir_lowering=False, debug=False, num_devices=E)
    dt = mybir.dt
    xT = nc.dram_tensor("xT", [D, C], dt.bfloat16, kind="ExternalInput")
    wgu = nc.dram_tensor("wguT", [D, F2], dt.bfloat16, kind="ExternalInput")
    wd = nc.dram_tensor("wdT", [F, D], dt.bfloat16, kind="ExternalInput")
    outT = nc.dram_tensor("outT", [D, C], dt.float32, kind="ExternalOutput")

    with TileContext(nc) as tc, ExitStack() as ctx:
        wpool = ctx.enter_context(tc.tile_pool(name="weights", bufs=1))
        wgu_sb = wpool.tile([128, KD, F2], dt.bfloat16, tag="wgu")
        nc.sync.dma_start(wgu_sb[:], wgu.rearrange("(k p) f -> p k f", p=128))
        wd_sb = wpool.tile([128, KF, D], dt.bfloat16, tag="wd")
        nc.sync.dma_start(wd_sb[:], wd.rearrange("(k p) f -> p k f", p=128))

        xpool = ctx.enter_context(tc.tile_pool(name="x", bufs=2))
        hpool = ctx.enter_context(tc.tile_pool(name="h", bufs=2))
        spool = ctx.enter_context(tc.tile_pool(name="silu", bufs=4))
        opool = ctx.enter_context(tc.tile_pool(name="o", bufs=4))
        pg = ctx.enter_context(tc.tile_pool(name="pg", bufs=2, space="PSUM"))
        pu = ctx.enter_context(tc.tile_pool(name="pu", bufs=2, space="PSUM"))
        po = ctx.enter_context(tc.tile_pool(name="po", bufs=2, space="PSUM"))

        xT_r = xT.rearrange("(k p) t -> p k t", p=128)
        outT_r = outT.rearrange("(m p) t -> p m t", p=128)

        def body():
            for n0 in [i for _ in range(repeats) for i in range(0, C, NT)]:
                nt = min(NT, C - n0)
                x_sb = xpool.tile([128, KD, NT], dt.bfloat16, tag="x")
                nc.sync.dma_start(x_sb[:, :, :nt], xT_r[:, :, n0:n0 + nt])
                h_sb = hpool.tile([128, KF, NT], dt.bfloat16, tag="h")
                for mp in range(KF):
                    psg = pg.tile([128, NT], dt.float32, tag="pg")
                    for k in range(KD):
                        nc.tensor.matmul(
                            psg[:, :nt], lhsT=wgu_sb[:, k, ts(mp, 128)],
                            rhs=x_sb[:, k, :nt], start=(k == 0), stop=(k == KD - 1))
                    psu = pu.tile([128, NT], dt.float32, tag="pu")
                    for k in range(KD):
                        nc.tensor.matmul(
                            psu[:, :nt], lhsT=wgu_sb[:, k, ds(F + mp * 128, 128)],
                            rhs=x_sb[:, k, :nt], start=(k == 0), stop=(k == KD - 1))
                    sil = spool.tile([128, NT], dt.bfloat16, tag="sil")
                    nc.scalar.activation(sil[:, :nt], psg[:, :nt],
                                         mybir.ActivationFunctionType.Silu)
                    nc.vector.tensor_mul(h_sb[:, mp, :nt], sil[:, :nt], psu[:, :nt])
                for m in range(MD):
                    pso = po.tile([128, NT], dt.float32, tag="po")
                    for k in range(KF):
                        nc.tensor.matmul(
                            pso[:, :nt], lhsT=wd_sb[:, k, ts(m, 128)],
                            rhs=h_sb[:, k, :nt], start=(k == 0), stop=(k == KF - 1))
                    o_sb = opool.tile([128, NT], dt.float32, tag="o")
                    nc.vector.tensor_copy(o_sb[:, :nt], pso[:, :nt])
                    nc.sync.dma_start(outT_r[:, m, n0:n0 + nt], o_sb[:, :nt])

        if hw_loop:
            with tc.For_i(0, hw_loop, 1):
                body()
        else:
            body()
    nc.finalize()
    return nc


def build_nc_wide(C, hw_loop=0):
    """Variant: 1024-token compute chunks with [128,1024] PSUM tiles.

    - halves ACT/DVE eviction instruction count (wide silu/mul)
    - consecutive matmuls share the same lhsT (LDW dedup opportunity)
    - PSUM banks: pg 2x2 + pu 1x2 + po 2x1 = 8
    """
    nc = bacc.Bacc("TRN2", target_bir_lowering=False, debug=False, num_devices=E)
    dt = mybir.dt
    NW = 1024
    xT = nc.dram_tensor("xT", [D, C], dt.bfloat16, kind="ExternalInput")
    wgu = nc.dram_tensor("wguT", [D, F2], dt.bfloat16, kind="ExternalInput")
    wd = nc.dram_tensor("wdT", [F, D], dt.bfloat16, kind="ExternalInput")
    outT = nc.dram_tensor("outT", [D, C], dt.float32, kind="ExternalOutput")

    with TileContext(nc) as tc, ExitStack() as ctx:
        wpool = ctx.enter_context(tc.tile_pool(name="weights", bufs=1))
        wgu_sb = wpool.tile([128, KD, F2], dt.bfloat16, tag="wgu")
        nc.sync.dma_start(wgu_sb[:], wgu.rearrange("(k p) f -> p k f", p=128))
        wd_sb = wpool.tile([128, KF, D], dt.bfloat16, tag="wd")
        nc.sync.dma_start(wd_sb[:], wd.rearrange("(k p) f -> p k f", p=128))

        xpool = ctx.enter_context(tc.tile_pool(name="x", bufs=1))
        hpool = ctx.enter_context(tc.tile_pool(name="h", bufs=3))
        spool = ctx.enter_context(tc.tile_pool(name="silu", bufs=3))
        opool = ctx.enter_context(tc.tile_pool(name="o", bufs=4))
        pg = ctx.enter_context(tc.tile_pool(name="pg", bufs=2, space="PSUM"))
        pu = ctx.enter_context(tc.tile_pool(name="pu", bufs=1, space="PSUM"))
        po = ctx.enter_context(tc.tile_pool(name="po", bufs=2, space="PSUM"))

        xT_r = xT.rearrange("(k p) t -> p k t", p=128)
        outT_r = outT.rearrange("(m p) t -> p m t", p=128)

        def do_chunk(n0, nw):
            # nw tokens starting at n0; nw in {1024, C % 1024}
            nh = (nw + NT - 1) // NT  # h sub-chunks of <=512
            x_sb = xpool.tile([128, KD, NW], dt.bfloat16, tag="x")
            nc.sync.dma_start(x_sb[:, :, :nw], xT_r[:, :, n0:n0 + nw])
            h_sbs = [hpool.tile([128, KF, NT], dt.bfloat16, tag="h",
                                name=f"h_{n0}_{s}")
                     for s in range(nh)]
            for mp in range(KF):
                psg = pg.tile([128, NW], dt.float32, tag="pg")
                for k in range(KD):
                    for s in range(nh):
                        w = min(NT, nw - s * NT)
                        nc.tensor.matmul(
                            psg[:, s * NT:s * NT + w],
                            lhsT=wgu_sb[:, k, ts(mp, 128)],
                            rhs=x_sb[:, k, s * NT:s * NT + w],
                            start=(k == 0), stop=(k == KD - 1))
                psu = pu.tile([128, NW], dt.float32, tag="pu")
                for k in range(KD):
                    for s in range(nh):
                        w = min(NT, nw - s * NT)
                        nc.tensor.matmul(
                            psu[:, s * NT:s * NT + w],
                            lhsT=wgu_sb[:, k, ds(F + mp * 128, 128)],
                            rhs=x_sb[:, k, s * NT:s * NT + w],
                            start=(k == 0), stop=(k == KD - 1))
                sil = spool.tile([128, NW], dt.bfloat16, tag="sil")
                nc.scalar.activation(sil[:, :nw], psg[:, :nw],
                                     mybir.ActivationFunctionType.Silu)
                for s in range(nh):
                    w = min(NT, nw - s * NT)
                    nc.vector.tensor_mul(h_sbs[s][:, mp, :w],
                                         sil[:, s * NT:s * NT + w],
                                         psu[:, s * NT:s * NT + w])
            for m in range(MD):
                for s in range(nh):
                    w = min(NT, nw - s * NT)
                    pso = po.tile([128, NT], dt.float32, tag="po")
                    for k in range(KF):
                        nc.tensor.matmul(
                            pso[:, :w], lhsT=wd_sb[:, k, ts(m, 128)],
                            rhs=h_sbs[s][:, k, :w],
                            start=(k == 0), stop=(k == KF - 1))
                    o_sb = opool.tile([128, NT], dt.float32, tag="o")
                    nc.vector.tensor_copy(o_sb[:, :w], pso[:, :w])
                    nc.sync.dma_start(outT_r[:, m, n0 + s * NT:n0 + s * NT + w],
                                      o_sb[:, :w])

        def body():
            for n0 in range(0, C, NW):
                do_chunk(n0, min(NW, C - n0))

        if hw_loop:
            with tc.For_i(0, hw_loop, 1):
                body()
        else:
            body()
    nc.finalize()
    return nc


def get_nc(C):
    # build_nc_v3: consumption-ordered weight streaming + double-buffered x.
    # TimelineSim: 342 us vs 365 us for build_nc_big (PE busy 337.5 us is the
    # bf16 streaming floor; v3 removes most cold-start DMA stall).
    if C not in _nc_cache:
        _nc_cache[C] = build_nc_v3(C)
    return _nc_cache[C]


def build_nc_v3(C, hw_loop=0, preload_weights=False, repeats=1):
    """1536-token chunks like build_nc_big, plus:

    - weight DMAs split [128, 512] per (k, f-block) and emitted in
      consumption order (gate f-blocks k-major, then up, then down),
      interleaved BEHIND the first chunk's x tiles, so the first matmul
      group unblocks after ~0.5 MB and the DMA stream stays ahead of the
      PE's weight-consumption order throughout.
    - xpool bufs=2: chunk n+1's x streams during chunk n's compute.
    - preload_weights: emit weight DMAs outside the hw_loop body (timing
      builds only) so a looped run models the single-shot steady state
      instead of re-streaming 17 MB of weights every rep.
    """
    nc = bacc.Bacc("TRN2", target_bir_lowering=False, debug=False, num_devices=E)
    dt = mybir.dt
    NB = 1536
    FB = 512  # weight f-block DMA granularity
    xT = nc.dram_tensor("xT", [D, C], dt.float16, kind="ExternalInput")
    wgu = nc.dram_tensor("wguT", [D, F2], dt.float16, kind="ExternalInput")
    wd = nc.dram_tensor("wdT", [F, D], dt.float16, kind="ExternalInput")
    outT = nc.dram_tensor("outT", [D, C], dt.float32, kind="ExternalOutput")

    with TileContext(nc) as tc, ExitStack() as ctx:
        wpool = ctx.enter_context(tc.tile_pool(name="weights", bufs=1))
        wgu_k = [wpool.tile([128, F2], dt.float16, tag=f"wgu{k}",
                            name=f"wgu{k}") for k in range(KD)]
        wd_k = [wpool.tile([128, D], dt.float16, tag=f"wd{k}",
                           name=f"wd{k}") for k in range(KF)]

        xpool = ctx.enter_context(tc.tile_pool(name="x", bufs=2))
        ghpool = ctx.enter_context(tc.tile_pool(name="gh", bufs=1))
        opool = ctx.enter_context(tc.tile_pool(name="o", bufs=4))
        pp = ctx.enter_context(tc.tile_pool(name="pp", bufs=2, space="PSUM"))
        po = ctx.enter_context(tc.tile_pool(name="po", bufs=2, space="PSUM"))

        xT_r = xT.rearrange("(k p) t -> p k t", p=128)
        outT_r = outT.rearrange("(m p) t -> p m t", p=128)

        def emit_weight_dmas(first_x=None):
            # gate f-blocks (cols 0..F), k-major inside each block, then up
            # (cols F..2F), then down weights; the first chunk's x k-tiles
            # lead the stream so the first gate k-chain isn't x-starved.
            if first_x is not None:
                # interleave G0's k-tiles with x's k-tiles, weight first:
                # LDW k only waits on its 128 KB weight block while the
                # matmul additionally waits on the 145 KB x k-tile.
                # (512-col blocks: 1 KB contiguous lines — under 512 B the
                # DMA engines pay a 2x latency multiplier.)
                x_sb, n0, nw = first_x
                for k in range(KD):
                    nc.sync.dma_start(wgu_k[k][:, 0:FB],
                                      wgu[k * 128:(k + 1) * 128, 0:FB])
                    nc.sync.dma_start(x_sb[:, k, :nw], xT_r[:, k, n0:n0 + nw])
                fb_start = FB
            else:
                fb_start = 0
            for fb in range(fb_start, F2, FB):
                for k in range(KD):
                    nc.sync.dma_start(wgu_k[k][:, fb:fb + FB],
                                      wgu[k * 128:(k + 1) * 128, fb:fb + FB])
            for k in range(KF):
                nc.sync.dma_start(wd_k[k][:], wd[k * 128:(k + 1) * 128, :])

        def slices(nw):
            return [(s, min(NT, nw - s)) for s in range(0, nw, NT)]

        def do_chunk(n0, nw, x_pre=None, is_last=False):
            if x_pre is None:
                x_sb = xpool.tile([128, KD, NB], dt.float16, tag="x")
                for k in range(KD):
                    nc.sync.dma_start(x_sb[:, k, :nw], xT_r[:, k, n0:n0 + nw])
            else:
                x_sb = x_pre
            gh = ghpool.tile([128, KF, NB], dt.float16, tag="gh")
            for phase in (0, 1):  # 0: gate+silu, 1: up+mul-in-place
                for mp in range(KF):
                    ps = pp.tile([128, NB], dt.float32, tag="pp",
                                 name=f"ps_{n0}_{phase}_{mp}")
                    f0 = mp * 128 if phase == 0 else F + mp * 128
                    for k in range(KD):
                        for s, w in slices(nw):
                            nc.tensor.matmul(
                                ps[:, s:s + w],
                                lhsT=wgu_k[k][:, ds(f0, 128)],
                                rhs=x_sb[:, k, s:s + w],
                                start=(k == 0), stop=(k == KD - 1))
                    if phase == 0:
                        nc.scalar.activation(gh[:, mp, :nw], ps[:, :nw],
                                             mybir.ActivationFunctionType.Silu)
                    else:
                        nc.vector.tensor_mul(gh[:, mp, :nw], gh[:, mp, :nw],
                                             ps[:, :nw])
            for m in range(MD):
                for s, w in slices(nw):
                    final = is_last and m == MD - 1 and s + w >= nw
                    # final psum group: two half-width chains so the last
                    # copy+DMA pipelines behind the second chain's matmuls
                    parts = [(s, w - w // 2), (s + w - w // 2, w // 2)] \
                        if final and w > 128 else [(s, w)]
                    for sp, wp in parts:
                        pso = po.tile([128, NT], dt.float32, tag="po",
                                      name=f"pso_{n0}_{m}_{sp}")
                        for k in range(KF):
                            nc.tensor.matmul(
                                pso[:, :wp], lhsT=wd_k[k][:, ts(m, 128)],
                                rhs=gh[:, k, sp:sp + wp],
                                start=(k == 0), stop=(k == KF - 1))
                        o_sb = opool.tile([128, NT], dt.float32, tag="o",
                                          name=f"o_{n0}_{m}_{sp}")
                        nc.vector.tensor_copy(o_sb[:, :wp], pso[:, :wp])
                        nc.sync.dma_start(outT_r[:, m, n0 + sp:n0 + sp + wp],
                                          o_sb[:, :wp])

        # smallest chunk first: the cold-start stall is gated on the first
        # chunk's x DMA, so lead with the cheapest one — but the first chunk
        # must stay big enough that its gate+up compute covers the 8.4 MB
        # gate/up weight stream (>= ~400 tokens; sim-validated optimum).
        chunks = [(n0, min(NB, C - n0)) for n0 in range(0, C, NB)]
        chunks.sort(key=lambda c: c[1])

        def body(first):
            for i, (n0, nw) in enumerate(chunks):
                last = i == len(chunks) - 1
                if i == 0 and first:
                    x_sb = xpool.tile([128, KD, NB], dt.float16, tag="x")
                    emit_weight_dmas(first_x=(x_sb, n0, nw))
                    do_chunk(n0, nw, x_pre=x_sb, is_last=last)
                else:
                    do_chunk(n0, nw, is_last=last)

        if hw_loop:
            if preload_weights:
                emit_weight_dmas()
            with tc.For_i(0, hw_loop, 1):
                body(first=not preload_weights)
        elif repeats > 1:
            # unrolled steady-state body (TimelineSim can't branch): weights
            # once, then the x+compute+out body repeated
            emit_weight_dmas()
            for _ in range(repeats):
                body(first=False)
        else:
            body(first=True)
    nc.finalize()
    return nc


def build_nc_big(C, hw_loop=0):
    """Variant: 1536-token chunks ([128,1536] 3-bank PSUM tiles).

    Streams 3x512 tokens per weight load (LDW count 1920 -> ~768), evicts
    gate via silu into a chunk-resident SBUF tensor, then multiplies the up
    projection into it in place. PSUM: pp 2x3 + po 2x1 = 8 banks.
    """
    nc = bacc.Bacc("TRN2", target_bir_lowering=False, debug=False, num_devices=E)
    dt = mybir.dt
    NB = 1536
    xT = nc.dram_tensor("xT", [D, C], dt.float16, kind="ExternalInput")
    wgu = nc.dram_tensor("wguT", [D, F2], dt.float16, kind="ExternalInput")
    wd = nc.dram_tensor("wdT", [F, D], dt.float16, kind="ExternalInput")
    outT = nc.dram_tensor("outT", [D, C], dt.float32, kind="ExternalOutput")

    with TileContext(nc) as tc, ExitStack() as ctx:
        # per-k weight tiles with separate DMAs; the first chunk's x tiles
        # are DMA'd BEFORE the weights (see do_chunk) so the PE's first
        # matmul group is gated on ~4 MB, not the full 16 MB input set.
        wpool = ctx.enter_context(tc.tile_pool(name="weights", bufs=1))
        wgu_k = [wpool.tile([128, F2], dt.float16, tag=f"wgu{k}",
                            name=f"wgu{k}") for k in range(KD)]
        wd_k = [wpool.tile([128, D], dt.float16, tag=f"wd{k}",
                           name=f"wd{k}") for k in range(KF)]

        xpool = ctx.enter_context(tc.tile_pool(name="x", bufs=1))
        ghpool = ctx.enter_context(tc.tile_pool(name="gh", bufs=1))
        opool = ctx.enter_context(tc.tile_pool(name="o", bufs=4))
        pp = ctx.enter_context(tc.tile_pool(name="pp", bufs=2, space="PSUM"))
        po = ctx.enter_context(tc.tile_pool(name="po", bufs=2, space="PSUM"))

        xT_r = xT.rearrange("(k p) t -> p k t", p=128)
        outT_r = outT.rearrange("(m p) t -> p m t", p=128)

        def slices(nw):
            return [(s, min(NT, nw - s)) for s in range(0, nw, NT)]

        def do_chunk(n0, nw):
            x_sb = xpool.tile([128, KD, NB], dt.float16, tag="x")
            if first[0]:
                first[0] = False
                # interleave x and gate/up weight k-tiles so the first
                # matmul group's operands stream in consumption order
                for k in range(KD):
                    nc.sync.dma_start(x_sb[:, k, :nw], xT_r[:, k, n0:n0 + nw])
                    # first 512 f-columns land first so the k-th LDW of the
                    # first gate group unblocks after ~0.4 MB, not 1 MB
                    nc.sync.dma_start(wgu_k[k][:, :NT],
                                      wgu[k * 128:(k + 1) * 128, :NT])
                    nc.sync.dma_start(wgu_k[k][:, NT:],
                                      wgu[k * 128:(k + 1) * 128, NT:])
                for k in range(KF):
                    nc.sync.dma_start(wd_k[k][:], wd[k * 128:(k + 1) * 128, :])
            else:
                for k in range(KD):
                    nc.sync.dma_start(x_sb[:, k, :nw], xT_r[:, k, n0:n0 + nw])
            gh = ghpool.tile([128, KF, NB], dt.float16, tag="gh")
            for phase in (0, 1):  # 0: gate+silu, 1: up+mul-in-place
                for mp in range(KF):
                    ps = pp.tile([128, NB], dt.float32, tag="pp",
                                 name=f"ps_{n0}_{phase}_{mp}")
                    f0 = mp * 128 if phase == 0 else F + mp * 128
                    for k in range(KD):
                        for s, w in slices(nw):
                            nc.tensor.matmul(
                                ps[:, s:s + w],
                                lhsT=wgu_k[k][:, ds(f0, 128)],
                                rhs=x_sb[:, k, s:s + w],
                                start=(k == 0), stop=(k == KD - 1))
                    if phase == 0:
                        nc.scalar.activation(gh[:, mp, :nw], ps[:, :nw],
                                             mybir.ActivationFunctionType.Silu)
                    else:
                        nc.vector.tensor_mul(gh[:, mp, :nw], gh[:, mp, :nw],
                                             ps[:, :nw])
            for m in range(MD):
                for s, w in slices(nw):
                    pso = po.tile([128, NT], dt.float32, tag="po",
                                  name=f"pso_{n0}_{m}_{s}")
                    for k in range(KF):
                        nc.tensor.matmul(
                            pso[:, :w], lhsT=wd_k[k][:, ts(m, 128)],
                            rhs=gh[:, k, s:s + w],
                            start=(k == 0), stop=(k == KF - 1))
                    o_sb = opool.tile([128, NT], dt.float32, tag="o",
                                      name=f"o_{n0}_{m}_{s}")
                    nc.vector.tensor_copy(o_sb[:, :w], pso[:, :w])
                    nc.sync.dma_start(outT_r[:, m, n0 + s:n0 + s + w],
                                      o_sb[:, :w])

        first = [True]

        def body():
            # smallest chunk first: the cold-start stall is gated on the
            # first chunk's x DMA, so lead with the cheapest one
            chunks = [(n0, min(NB, C - n0)) for n0 in range(0, C, NB)]
            chunks.sort(key=lambda c: c[1])
            for n0, nw in chunks:
                do_chunk(n0, nw)



        if hw_loop:
            with tc.For_i(0, hw_loop, 1):
                body()
        else:
            body()
    nc.finalize()
    return nc


def route(x, expert_indices):
    """Sort tokens by expert; return (order, counts, capacity C)."""
    idx = np.asarray(expert_indices)
    order = np.argsort(idx, kind="stable")
    counts = np.bincount(idx, minlength=E).astype(np.int64)
    C = max(NT, int(-(-counts.max() // 8) * 8))
    return order, counts, C


def make_in_maps(x, expert_indices, gate_up_weight, down_weight):
    order, counts, C = route(x, expert_indices)
    x_sorted = np.asarray(x, dtype=np.float32)[order]
    offs = np.zeros(E + 1, dtype=np.int64)
    np.cumsum(counts, out=offs[1:])
    wguT = np.ascontiguousarray(
        np.transpose(np.asarray(gate_up_weight), (0, 2, 1))).astype(F16)
    wdT = np.ascontiguousarray(
        np.transpose(np.asarray(down_weight), (0, 2, 1))).astype(F16)
    in_maps = []
    for e in range(E):
        xe = np.zeros((C, D), dtype=np.float32)
        xe[: counts[e]] = x_sorted[offs[e]: offs[e + 1]]
        in_maps.append({
            "xT": np.ascontiguousarray(xe.T).astype(F16),
            "wguT": wguT[e],
            "wdT": wdT[e],
        })
    return in_maps, order, counts, C


def assemble_output(results, order, counts):
    T = int(counts.sum())
    out = np.empty((T, D), dtype=np.float32)
    offs = np.zeros(E + 1, dtype=np.int64)
    np.cumsum(counts, out=offs[1:])
    sorted_out = np.empty((T, D), dtype=np.float32)
    for e in range(E):
        sorted_out[offs[e]: offs[e + 1]] = results[e]["outT"].T[: counts[e]]
    out[order] = sorted_out
    return out


def kernel(x, expert_indices, gate_up_weight, down_weight):
    in_maps, order, counts, C = make_in_maps(
        x, expert_indices, gate_up_weight, down_weight)
    nc = get_nc(C)
    res = run_bass_kernel_spmd(nc, in_maps, core_ids=list(range(E)))
    return assemble_output(res.results, order, counts)



# revision 19
# speedup vs baseline: 1.3415x; 1.3415x over previous
"""MoE batched-experts kernel for Trainium2 (8 NeuronCores, expert-parallel).

Contract: kernel(**inputs) takes the FULL unsharded inputs
  x:              [T, D]      float32   (T=16384, D=1024)
  expert_indices: [T]         int32     (values in [0, 8))
  gate_up_weight: [E, 2F, D]  float32   (E=8, F=2048)
  down_weight:    [E, D, F]   float32
and returns the FULL output [T, D] float32:
  per token t with expert e: h = silu(x @ gu[e,:F].T) * (x @ gu[e,F:].T);
  out = h @ dw[e].T

Strategy: expert-parallel. The host routes (sorts) tokens by expert, pads
each expert's group to a common capacity C (max count rounded up to 8), and
core e runs a dense FFN for expert e on its token group. All operands are
pre-transposed / pre-cast to fp16 on the host (same PE rate as bf16, 8x the
mantissa: rel-err ~6e-4 vs ~4.5e-3) so the device kernel is pure matmul +
silu*mul with no on-chip transposes:
  core e computes outT = w_d @ (silu(w_gT.T @ xT) * (w_uT.T @ xT))
with xT [D, C], producing outT [D, C] fp32; the host transposes back and
unpermutes.
"""

import numpy as np
import ml_dtypes

import concourse.bass as bass
import concourse.mybir as mybir
from concourse import bacc
from concourse.tile import TileContext
from concourse.bass import ts, ds
from concourse.bass_utils import run_bass_kernel_spmd
from contextlib import ExitStack

BF16 = ml_dtypes.bfloat16
F16 = np.float16  # operand dtype for matmuls: same PE rate as bf16, 8x mantissa

D = 1024      # d_model
F = 2048      # d_ff
F2 = 2 * F    # gate+up
E = 8         # experts == cores
KD = D // 128   # 8  k-tiles over d_model
KF = F // 128   # 16 k-tiles over d_ff
MD = D // 128   # 8  m-tiles over d_model (output)
NT = 512        # token chunk (one PSUM bank at fp32)

_nc_cache = {}


def build_nc(C, repeats=1, hw_loop=0):
    """Build the per-core dense-FFN Bass program for token capacity C.

    repeats>1 re-emits the whole compute body (unrolled); hw_loop>0 wraps the
    body in a hardware For_i loop. Both are timing aids: slope of time vs
    repetition count isolates true exec time from dispatch overhead."""
    nc = bacc.Bacc("TRN2", target_bir_lowering=False, debug=False, num_devices=E)
    dt = mybir.dt
    xT = nc.dram_tensor("xT", [D, C], dt.bfloat16, kind="ExternalInput")
    wgu = nc.dram_tensor("wguT", [D, F2], dt.bfloat16, kind="ExternalInput")
    wd = nc.dram_tensor("wdT", [F, D], dt.bfloat16, kind="ExternalInput")
    outT = nc.dram_tensor("outT", [D, C], dt.float32, kind="ExternalOutput")

    with TileContext(nc) as tc, ExitStack() as ctx:
        wpool = ctx.enter_context(tc.tile_pool(name="weights", bufs=1))
        wgu_sb = wpool.tile([128, KD, F2], dt.bfloat16, tag="wgu")
        nc.sync.dma_start(wgu_sb[:], wgu.rearrange("(k p) f -> p k f", p=128))
        wd_sb = wpool.tile([128, KF, D], dt.bfloat16, tag="wd")
        nc.sync.dma_start(wd_sb[:], wd.rearrange("(k p) f -> p k f", p=128))

        xpool = ctx.enter_context(tc.tile_pool(name="x", bufs=2))
        hpool = ctx.enter_context(tc.tile_pool(name="h", bufs=2))
        spool = ctx.enter_context(tc.tile_pool(name="silu", bufs=4))
        opool = ctx.enter_context(tc.tile_pool(name="o", bufs=4))
        pg = ctx.enter_context(tc.tile_pool(name="pg", bufs=2, space="PSUM"))
        pu = ctx.enter_context(tc.tile_pool(name="pu", bufs=2, space="PSUM"))
        po = ctx.enter_context(tc.tile_pool(name="po", bufs=2, space="PSUM"))

        xT_r = xT.rearrange("(k p) t -> p k t", p=128)
        outT_r = outT.rearrange("(m p) t -> p m t", p=128)

        def body():
            for n0 in [i for _ in range(repeats) for i in range(0, C, NT)]:
                nt = min(NT, C - n0)
                x_sb = xpool.tile([128, KD, NT], dt.bfloat16, tag="x")
                nc.sync.dma_start(x_sb[:, :, :nt], xT_r[:, :, n0:n0 + nt])
                h_sb = hpool.tile([128, KF, NT], dt.bfloat16, tag="h")
                for mp in range(KF):
                    psg = pg.tile([128, NT], dt.float32, tag="pg")
                    for k in range(KD):
                        nc.tensor.matmul(
                            psg[:, :nt], lhsT=wgu_sb[:, k, ts(mp, 128)],
                            rhs=x_sb[:, k, :nt], start=(k == 0), stop=(k == KD - 1))
                    psu = pu.tile([128, NT], dt.float32, tag="pu")
                    for k in range(KD):
                        nc.tensor.matmul(
                            psu[:, :nt], lhsT=wgu_sb[:, k, ds(F + mp * 128, 128)],
                            rhs=x_sb[:, k, :nt], start=(k == 0), stop=(k == KD - 1))
                    sil = spool.tile([128, NT], dt.bfloat16, tag="sil")
                    nc.scalar.activation(sil[:, :nt], psg[:, :nt],
                                         mybir.ActivationFunctionType.Silu)
                    nc.vector.tensor_mul(h_sb[:, mp, :nt], sil[:, :nt], psu[:, :nt])
                for m in range(MD):
                    pso = po.tile([128, NT], dt.float32, tag="po")
                    for k in range(KF):
                        nc.tensor.matmul(
                            pso[:, :nt], lhsT=wd_sb[:, k, ts(m, 128)],
                            rhs=h_sb[:, k, :nt], start=(k == 0), stop=(k == KF - 1))
                    o_sb = opool.tile([128, NT], dt.float32, tag="o")
                    nc.vector.tensor_copy(o_sb[:, :nt], pso[:, :nt])
                    nc.sync.dma_start(outT_r[:, m, n0:n0 + nt], o_sb[:, :nt])

        if hw_loop:
            with tc.For_i(0, hw_loop, 1):
                body()
        else:
            body()
    nc.finalize()
    return nc


def build_nc_wide(C, hw_loop=0):
    """Variant: 1024-token compute chunks with [128,1024] PSUM tiles.

    - halves ACT/DVE eviction instruction count (wide silu/mul)
    - consecutive matmuls share the same lhsT (LDW dedup opportunity)
    - PSUM banks: pg 2x2 + pu 1x2 + po 2x1 = 8
    """
    nc = bacc.Bacc("TRN2", target_bir_lowering=False, debug=False, num_devices=E)
    dt = mybir.dt
    NW = 1024
    xT = nc.dram_tensor("xT", [D, C], dt.bfloat16, kind="ExternalInput")
    wgu = nc.dram_tensor("wguT", [D, F2], dt.bfloat16, kind="ExternalInput")
    wd = nc.dram_tensor("wdT", [F, D], dt.bfloat16, kind="ExternalInput")
    outT = nc.dram_tensor("outT", [D, C], dt.float32, kind="ExternalOutput")

    with TileContext(nc) as tc, ExitStack() as ctx:
        wpool = ctx.enter_context(tc.tile_pool(name="weights", bufs=1))
        wgu_sb = wpool.tile([128, KD, F2], dt.bfloat16, tag="wgu")
        nc.sync.dma_start(wgu_sb[:], wgu.rearrange("(k p) f -> p k f", p=128))
        wd_sb = wpool.tile([128, KF, D], dt.bfloat16, tag="wd")
        nc.sync.dma_start(wd_sb[:], wd.rearrange("(k p) f -> p k f", p=128))

        xpool = ctx.enter_context(tc.tile_pool(name="x", bufs=1))
        hpool = ctx.enter_context(tc.tile_pool(name="h", bufs=3))
        spool = ctx.enter_context(tc.tile_pool(name="silu", bufs=3))
        opool = ctx.enter_context(tc.tile_pool(name="o", bufs=4))
        pg = ctx.enter_context(tc.tile_pool(name="pg", bufs=2, space="PSUM"))
        pu = ctx.enter_context(tc.tile_pool(name="pu", bufs=1, space="PSUM"))
        po = ctx.enter_context(tc.tile_pool(name="po", bufs=2, space="PSUM"))

        xT_r = xT.rearrange("(k p) t -> p k t", p=128)
        outT_r = outT.rearrange("(m p) t -> p m t", p=128)

        def do_chunk(n0, nw):
            # nw tokens starting at n0; nw in {1024, C % 1024}
            nh = (nw + NT - 1) // NT  # h sub-chunks of <=512
            x_sb = xpool.tile([128, KD, NW], dt.bfloat16, tag="x")
            nc.sync.dma_start(x_sb[:, :, :nw], xT_r[:, :, n0:n0 + nw])
            h_sbs = [hpool.tile([128, KF, NT], dt.bfloat16, tag="h",
                                name=f"h_{n0}_{s}")
                     for s in range(nh)]
            for mp in range(KF):
                psg = pg.tile([128, NW], dt.float32, tag="pg")
                for k in range(KD):
                    for s in range(nh):
                        w = min(NT, nw - s * NT)
                        nc.tensor.matmul(
                            psg[:, s * NT:s * NT + w],
                            lhsT=wgu_sb[:, k, ts(mp, 128)],
                            rhs=x_sb[:, k, s * NT:s * NT + w],
                            start=(k == 0), stop=(k == KD - 1))
                psu = pu.tile([128, NW], dt.float32, tag="pu")
                for k in range(KD):
                    for s in range(nh):
                        w = min(NT, nw - s * NT)
                        nc.tensor.matmul(
                            psu[:, s * NT:s * NT + w],
                            lhsT=wgu_sb[:, k, ds(F + mp * 128, 128)],
                            rhs=x_sb[:, k, s * NT:s * NT + w],
                            start=(k == 0), stop=(k == KD - 1))
                sil = spool.tile([128, NW], dt.bfloat16, tag="sil")
                nc.scalar.activation(sil[:, :nw], psg[:, :nw],
                                     mybir.ActivationFunctionType.Silu)
                for s in range(nh):
                    w = min(NT, nw - s * NT)
                    nc.vector.tensor_mul(h_sbs[s][:, mp, :w],
                                         sil[:, s * NT:s * NT + w],
                                         psu[:, s * NT:s * NT + w])
            for m in range(MD):
                for s in range(nh):
                    w = min(NT, nw - s * NT)
                    pso = po.tile([128, NT], dt.float32, tag="po")
                    for k in range(KF):
                        nc.tensor.matmul(
                            pso[:, :w], lhsT=wd_sb[:, k, ts(m, 128)],
                            rhs=h_sbs[s][:, k, :w],
                            start=(k == 0), stop=(k == KF - 1))
                    o_sb = opool.tile([128, NT], dt.float32, tag="o")
                    nc.vector.tensor_copy(o_sb[:, :w], pso[:, :w])
                    nc.sync.dma_start(outT_r[:, m, n0 + s * NT:n0 + s * NT + w],
                                      o_sb[:, :w])

        def body():
            for n0 in range(0, C, NW):
                do_chunk(n0, min(NW, C - n0))

        if hw_loop:
            with tc.For_i(0, hw_loop, 1):
                body()
        else:
            body()
    nc.finalize()
    return nc


def get_nc(C):
    # build_nc_v3: consumption-ordered weight streaming + double-buffered x.
    # TimelineSim: 342 us vs 365 us for build_nc_big (PE busy 337.5 us is the
    # bf16 streaming floor; v3 removes most cold-start DMA stall).
    if C not in _nc_cache:
        _nc_cache[C] = build_nc_v3(C)
    return _nc_cache[C]


def build_nc_v3(C, hw_loop=0, preload_weights=False, repeats=1,
                out_f16=True):
    """1536-token chunks like build_nc_big, plus:

    - weight DMAs split [128, 512] per (k, f-block) and emitted in
      consumption order (gate f-blocks k-major, then up, then down),
      interleaved BEHIND the first chunk's x tiles, so the first matmul
      group unblocks after ~0.5 MB and the DMA stream stays ahead of the
      PE's weight-consumption order throughout.
    - xpool bufs=2: chunk n+1's x streams during chunk n's compute.
    - preload_weights: emit weight DMAs outside the hw_loop body (timing
      builds only) so a looped run models the single-shot steady state
      instead of re-streaming 17 MB of weights every rep.
    """
    nc = bacc.Bacc("TRN2", target_bir_lowering=False, debug=False, num_devices=E)
    dt = mybir.dt
    NB = 1536
    FB = 512  # weight f-block DMA granularity
    xT = nc.dram_tensor("xT", [D, C], dt.float16, kind="ExternalInput")
    wgu = nc.dram_tensor("wguT", [D, F2], dt.float16, kind="ExternalInput")
    wd = nc.dram_tensor("wdT", [F, D], dt.float16, kind="ExternalInput")
    out_dt = dt.float16 if out_f16 else dt.float32
    outT = nc.dram_tensor("outT", [D, C], out_dt, kind="ExternalOutput")

    with TileContext(nc) as tc, ExitStack() as ctx:
        wpool = ctx.enter_context(tc.tile_pool(name="weights", bufs=1))
        wgu_k = [wpool.tile([128, F2], dt.float16, tag=f"wgu{k}",
                            name=f"wgu{k}") for k in range(KD)]
        wd_k = [wpool.tile([128, D], dt.float16, tag=f"wd{k}",
                           name=f"wd{k}") for k in range(KF)]

        xpool = ctx.enter_context(tc.tile_pool(name="x", bufs=2))
        ghpool = ctx.enter_context(tc.tile_pool(name="gh", bufs=1))
        opool = ctx.enter_context(tc.tile_pool(name="o", bufs=4))
        pp = ctx.enter_context(tc.tile_pool(name="pp", bufs=2, space="PSUM"))
        po = ctx.enter_context(tc.tile_pool(name="po", bufs=2, space="PSUM"))

        xT_r = xT.rearrange("(k p) t -> p k t", p=128)
        outT_r = outT.rearrange("(m p) t -> p m t", p=128)

        def emit_weight_dmas(first_x=None):
            # gate f-blocks (cols 0..F), k-major inside each block, then up
            # (cols F..2F), then down weights; the first chunk's x k-tiles
            # lead the stream so the first gate k-chain isn't x-starved.
            if first_x is not None:
                # interleave G0's k-tiles with x's k-tiles, weight first:
                # LDW k only waits on its 128 KB weight block while the
                # matmul additionally waits on the 145 KB x k-tile.
                # (512-col blocks: 1 KB contiguous lines — under 512 B the
                # DMA engines pay a 2x latency multiplier.)
                x_sb, n0, nw = first_x
                for k in range(KD):
                    nc.sync.dma_start(wgu_k[k][:, 0:FB],
                                      wgu[k * 128:(k + 1) * 128, 0:FB])
                    nc.sync.dma_start(x_sb[:, k, :nw], xT_r[:, k, n0:n0 + nw])
                fb_start = FB
            else:
                fb_start = 0
            for fb in range(fb_start, F2, FB):
                for k in range(KD):
                    nc.sync.dma_start(wgu_k[k][:, fb:fb + FB],
                                      wgu[k * 128:(k + 1) * 128, fb:fb + FB])
            for k in range(KF):
                nc.sync.dma_start(wd_k[k][:], wd[k * 128:(k + 1) * 128, :])

        def slices(nw):
            return [(s, min(NT, nw - s)) for s in range(0, nw, NT)]

        def do_chunk(n0, nw, x_pre=None, is_last=False):
            if x_pre is None:
                x_sb = xpool.tile([128, KD, NB], dt.float16, tag="x")
                for k in range(KD):
                    nc.sync.dma_start(x_sb[:, k, :nw], xT_r[:, k, n0:n0 + nw])
            else:
                x_sb = x_pre
            gh = ghpool.tile([128, KF, NB], dt.float16, tag="gh")
            for phase in (0, 1):  # 0: gate+silu, 1: up+mul-in-place
                for mp in range(KF):
                    ps = pp.tile([128, NB], dt.float32, tag="pp",
                                 name=f"ps_{n0}_{phase}_{mp}")
                    f0 = mp * 128 if phase == 0 else F + mp * 128
                    for k in range(KD):
                        for s, w in slices(nw):
                            nc.tensor.matmul(
                                ps[:, s:s + w],
                                lhsT=wgu_k[k][:, ds(f0, 128)],
                                rhs=x_sb[:, k, s:s + w],
                                start=(k == 0), stop=(k == KD - 1))
                    if phase == 0:
                        nc.scalar.activation(gh[:, mp, :nw], ps[:, :nw],
                                             mybir.ActivationFunctionType.Silu)
                    else:
                        nc.vector.tensor_mul(gh[:, mp, :nw], gh[:, mp, :nw],
                                             ps[:, :nw])
            for m in range(MD):
                for s, w in slices(nw):
                    final = is_last and m == MD - 1 and s + w >= nw
                    # final psum group: two half-width chains so the last
                    # copy+DMA pipelines behind the second chain's matmuls
                    parts = [(s, w - w // 2), (s + w - w // 2, w // 2)] \
                        if final and w > 128 else [(s, w)]
                    for sp, wp in parts:
                        pso = po.tile([128, NT], dt.float32, tag="po",
                                      name=f"pso_{n0}_{m}_{sp}")
                        for k in range(KF):
                            nc.tensor.matmul(
                                pso[:, :wp], lhsT=wd_k[k][:, ts(m, 128)],
                                rhs=gh[:, k, sp:sp + wp],
                                start=(k == 0), stop=(k == KF - 1))
                        o_sb = opool.tile([128, NT], out_dt, tag="o",
                                          name=f"o_{n0}_{m}_{sp}")
                        nc.vector.tensor_copy(o_sb[:, :wp], pso[:, :wp])
                        nc.sync.dma_start(outT_r[:, m, n0 + sp:n0 + sp + wp],
                                          o_sb[:, :wp])

        # smallest chunk first: the cold-start stall is gated on the first
        # chunk's x DMA, so lead with the cheapest one — but the first chunk
        # must stay big enough that its gate+up compute covers the 8.4 MB
        # gate/up weight stream (>= ~400 tokens; sim-validated optimum).
        chunks = [(n0, min(NB, C - n0)) for n0 in range(0, C, NB)]
        chunks.sort(key=lambda c: c[1])

        def body(first):
            for i, (n0, nw) in enumerate(chunks):
                last = i == len(chunks) - 1
                if i == 0 and first:
                    x_sb = xpool.tile([128, KD, NB], dt.float16, tag="x")
                    emit_weight_dmas(first_x=(x_sb, n0, nw))
                    do_chunk(n0, nw, x_pre=x_sb, is_last=last)
                else:
                    do_chunk(n0, nw, is_last=last)

        if hw_loop:
            if preload_weights:
                emit_weight_dmas()
            with tc.For_i(0, hw_loop, 1):
                body(first=not preload_weights)
        elif repeats > 1:
            # unrolled steady-state body (TimelineSim can't branch): weights
            # once, then the x+compute+out body repeated
            emit_weight_dmas()
            for _ in range(repeats):
                body(first=False)
        else:
            body(first=True)
    nc.finalize()
    return nc


def build_nc_big(C, hw_loop=0):
    """Variant: 1536-token chunks ([128,1536] 3-bank PSUM tiles).

    Streams 3x512 tokens per weight load (LDW count 1920 -> ~768), evicts
    gate via silu into a chunk-resident SBUF tensor, then multiplies the up
    projection into it in place. PSUM: pp 2x3 + po 2x1 = 8 banks.
    """
    nc = bacc.Bacc("TRN2", target_bir_lowering=False, debug=False, num_devices=E)
    dt = mybir.dt
    NB = 1536
    xT = nc.dram_tensor("xT", [D, C], dt.float16, kind="ExternalInput")
    wgu = nc.dram_tensor("wguT", [D, F2], dt.float16, kind="ExternalInput")
    wd = nc.dram_tensor("wdT", [F, D], dt.float16, kind="ExternalInput")
    outT = nc.dram_tensor("outT", [D, C], dt.float32, kind="ExternalOutput")

    with TileContext(nc) as tc, ExitStack() as ctx:
        # per-k weight tiles with separate DMAs; the first chunk's x tiles
        # are DMA'd BEFORE the weights (see do_chunk) so the PE's first
        # matmul group is gated on ~4 MB, not the full 16 MB input set.
        wpool = ctx.enter_context(tc.tile_pool(name="weights", bufs=1))
        wgu_k = [wpool.tile([128, F2], dt.float16, tag=f"wgu{k}",
                            name=f"wgu{k}") for k in range(KD)]
        wd_k = [wpool.tile([128, D], dt.float16, tag=f"wd{k}",
                           name=f"wd{k}") for k in range(KF)]

        xpool = ctx.enter_context(tc.tile_pool(name="x", bufs=1))
        ghpool = ctx.enter_context(tc.tile_pool(name="gh", bufs=1))
        opool = ctx.enter_context(tc.tile_pool(name="o", bufs=4))
        pp = ctx.enter_context(tc.tile_pool(name="pp", bufs=2, space="PSUM"))
        po = ctx.enter_context(tc.tile_pool(name="po", bufs=2, space="PSUM"))

        xT_r = xT.rearrange("(k p) t -> p k t", p=128)
        outT_r = outT.rearrange("(m p) t -> p m t", p=128)

        def slices(nw):
            return [(s, min(NT, nw - s)) for s in range(0, nw, NT)]

        def do_chunk(n0, nw):
            x_sb = xpool.tile([128, KD, NB], dt.float16, tag="x")
            if first[0]:
                first[0] = False
                # interleave x and gate/up weight k-tiles so the first
                # matmul group's operands stream in consumption order
                for k in range(KD):
                    nc.sync.dma_start(x_sb[:, k, :nw], xT_r[:, k, n0:n0 + nw])
                    # first 512 f-columns land first so the k-th LDW of the
                    # first gate group unblocks after ~0.4 MB, not 1 MB
                    nc.sync.dma_start(wgu_k[k][:, :NT],
                                      wgu[k * 128:(k + 1) * 128, :NT])
                    nc.sync.dma_start(wgu_k[k][:, NT:],
                                      wgu[k * 128:(k + 1) * 128, NT:])
                for k in range(KF):
                    nc.sync.dma_start(wd_k[k][:], wd[k * 128:(k + 1) * 128, :])
            else:
                for k in range(KD):
                    nc.sync.dma_start(x_sb[:, k, :nw], xT_r[:, k, n0:n0 + nw])
            gh = ghpool.tile([128, KF, NB], dt.float16, tag="gh")
            for phase in (0, 1):  # 0: gate+silu, 1: up+mul-in-place
                for mp in range(KF):
                    ps = pp.tile([128, NB], dt.float32, tag="pp",
                                 name=f"ps_{n0}_{phase}_{mp}")
                    f0 = mp * 128 if phase == 0 else F + mp * 128
                    for k in range(KD):
                        for s, w in slices(nw):
                            nc.tensor.matmul(
                                ps[:, s:s + w],
                                lhsT=wgu_k[k][:, ds(f0, 128)],
                                rhs=x_sb[:, k, s:s + w],
                                start=(k == 0), stop=(k == KD - 1))
                    if phase == 0:
                        nc.scalar.activation(gh[:, mp, :nw], ps[:, :nw],
                                             mybir.ActivationFunctionType.Silu)
                    else:
                        nc.vector.tensor_mul(gh[:, mp, :nw], gh[:, mp, :nw],
                                             ps[:, :nw])
            for m in range(MD):
                for s, w in slices(nw):
                    pso = po.tile([128, NT], dt.float32, tag="po",
                                  name=f"pso_{n0}_{m}_{s}")
                    for k in range(KF):
                        nc.tensor.matmul(
                            pso[:, :w], lhsT=wd_k[k][:, ts(m, 128)],
                            rhs=gh[:, k, s:s + w],
                            start=(k == 0), stop=(k == KF - 1))
                    o_sb = opool.tile([128, NT], dt.float32, tag="o",
                                      name=f"o_{n0}_{m}_{s}")
                    nc.vector.tensor_copy(o_sb[:, :w], pso[:, :w])
                    nc.sync.dma_start(outT_r[:, m, n0 + s:n0 + s + w],
                                      o_sb[:, :w])

        first = [True]

        def body():
            # smallest chunk first: the cold-start stall is gated on the
            # first chunk's x DMA, so lead with the cheapest one
            chunks = [(n0, min(NB, C - n0)) for n0 in range(0, C, NB)]
            chunks.sort(key=lambda c: c[1])
            for n0, nw in chunks:
                do_chunk(n0, nw)



        if hw_loop:
            with tc.For_i(0, hw_loop, 1):
                body()
        else:
            body()
    nc.finalize()
    return nc


def route(x, expert_indices):
    """Sort tokens by expert; return (order, counts, capacity C)."""
    idx = np.asarray(expert_indices)
    order = np.argsort(idx, kind="stable")
    counts = np.bincount(idx, minlength=E).astype(np.int64)
    C = max(NT, int(-(-counts.max() // 8) * 8))
    return order, counts, C


def make_in_maps(x, expert_indices, gate_up_weight, down_weight):
    order, counts, C = route(x, expert_indices)
    x_sorted = np.asarray(x, dtype=np.float32)[order]
    offs = np.zeros(E + 1, dtype=np.int64)
    np.cumsum(counts, out=offs[1:])
    wguT = np.ascontiguousarray(
        np.transpose(np.asarray(gate_up_weight), (0, 2, 1))).astype(F16)
    wdT = np.ascontiguousarray(
        np.transpose(np.asarray(down_weight), (0, 2, 1))).astype(F16)
    in_maps = []
    for e in range(E):
        xe = np.zeros((C, D), dtype=np.float32)
        xe[: counts[e]] = x_sorted[offs[e]: offs[e + 1]]
        in_maps.append({
            "xT": np.ascontiguousarray(xe.T).astype(F16),
            "wguT": wguT[e],
            "wdT": wdT[e],
        })
    return in_maps, order, counts, C


def assemble_output(results, order, counts):
    T = int(counts.sum())
    out = np.empty((T, D), dtype=np.float32)
    offs = np.zeros(E + 1, dtype=np.int64)
    np.cumsum(counts, out=offs[1:])
    sorted_out = np.empty((T, D), dtype=np.float32)
    for e in range(E):
        sorted_out[offs[e]: offs[e + 1]] = results[e]["outT"].T[: counts[e]]
    out[order] = sorted_out
    return out


def kernel(x, expert_indices, gate_up_weight, down_weight):
    in_maps, order, counts, C = make_in_maps(
        x, expert_indices, gate_up_weight, down_weight)
    nc = get_nc(C)
    res = run_bass_kernel_spmd(nc, in_maps, core_ids=list(range(E)))
    return assemble_output(res.results, order, counts)



# revision 26
# speedup vs baseline: 1.4706x; 1.0963x over previous
"""MoE batched-experts kernel for Trainium2 (8 NeuronCores, expert-parallel).

Contract: kernel(**inputs) takes the FULL unsharded inputs
  x:              [T, D]      float32   (T=16384, D=1024)
  expert_indices: [T]         int32     (values in [0, 8))
  gate_up_weight: [E, 2F, D]  float32   (E=8, F=2048)
  down_weight:    [E, D, F]   float32
and returns the FULL output [T, D] float32:
  per token t with expert e: h = silu(x @ gu[e,:F].T) * (x @ gu[e,F:].T);
  out = h @ dw[e].T

Strategy: expert-parallel. The host routes (sorts) tokens by expert, pads
each expert's group to a common capacity C (max count rounded up to 8), and
core e runs a dense FFN for expert e on its token group. All operands are
pre-transposed / pre-cast to fp16 on the host (same PE rate as bf16, 8x the
mantissa: rel-err ~6e-4 vs ~4.5e-3) so the device kernel is pure matmul +
silu*mul with no on-chip transposes:
  core e computes outT = w_d @ (silu(w_gT.T @ xT) * (w_uT.T @ xT))
with xT [D, C], producing outT [D, C] fp32; the host transposes back and
unpermutes.
"""

import numpy as np
import ml_dtypes

import concourse.bass as bass
import concourse.mybir as mybir
from concourse import bacc
from concourse.tile import TileContext
from concourse.bass import ts, ds
from concourse.bass_utils import run_bass_kernel_spmd
from contextlib import ExitStack

BF16 = ml_dtypes.bfloat16
F16 = np.float16  # operand dtype for matmuls: same PE rate as bf16, 8x mantissa

D = 1024      # d_model
F = 2048      # d_ff
F2 = 2 * F    # gate+up
E = 8         # experts == cores
KD = D // 128   # 8  k-tiles over d_model
KF = F // 128   # 16 k-tiles over d_ff
MD = D // 128   # 8  m-tiles over d_model (output)
NT = 512        # token chunk (one PSUM bank at fp32)

_nc_cache = {}


def build_nc(C, repeats=1, hw_loop=0):
    """Build the per-core dense-FFN Bass program for token capacity C.

    repeats>1 re-emits the whole compute body (unrolled); hw_loop>0 wraps the
    body in a hardware For_i loop. Both are timing aids: slope of time vs
    repetition count isolates true exec time from dispatch overhead."""
    nc = bacc.Bacc("TRN2", target_bir_lowering=False, debug=False, num_devices=E)
    dt = mybir.dt
    xT = nc.dram_tensor("xT", [D, C], dt.bfloat16, kind="ExternalInput")
    wgu = nc.dram_tensor("wguT", [D, F2], dt.bfloat16, kind="ExternalInput")
    wd = nc.dram_tensor("wdT", [F, D], dt.bfloat16, kind="ExternalInput")
    outT = nc.dram_tensor("outT", [D, C], dt.float32, kind="ExternalOutput")

    with TileContext(nc) as tc, ExitStack() as ctx:
        wpool = ctx.enter_context(tc.tile_pool(name="weights", bufs=1))
        wgu_sb = wpool.tile([128, KD, F2], dt.bfloat16, tag="wgu")
        nc.sync.dma_start(wgu_sb[:], wgu.rearrange("(k p) f -> p k f", p=128))
        wd_sb = wpool.tile([128, KF, D], dt.bfloat16, tag="wd")
        nc.sync.dma_start(wd_sb[:], wd.rearrange("(k p) f -> p k f", p=128))

        xpool = ctx.enter_context(tc.tile_pool(name="x", bufs=2))
        hpool = ctx.enter_context(tc.tile_pool(name="h", bufs=2))
        spool = ctx.enter_context(tc.tile_pool(name="silu", bufs=4))
        opool = ctx.enter_context(tc.tile_pool(name="o", bufs=4))
        pg = ctx.enter_context(tc.tile_pool(name="pg", bufs=2, space="PSUM"))
        pu = ctx.enter_context(tc.tile_pool(name="pu", bufs=2, space="PSUM"))
        po = ctx.enter_context(tc.tile_pool(name="po", bufs=2, space="PSUM"))

        xT_r = xT.rearrange("(k p) t -> p k t", p=128)
        outT_r = outT.rearrange("(m p) t -> p m t", p=128)

        def body():
            for n0 in [i for _ in range(repeats) for i in range(0, C, NT)]:
                nt = min(NT, C - n0)
                x_sb = xpool.tile([128, KD, NT], dt.bfloat16, tag="x")
                nc.sync.dma_start(x_sb[:, :, :nt], xT_r[:, :, n0:n0 + nt])
                h_sb = hpool.tile([128, KF, NT], dt.bfloat16, tag="h")
                for mp in range(KF):
                    psg = pg.tile([128, NT], dt.float32, tag="pg")
                    for k in range(KD):
                        nc.tensor.matmul(
                            psg[:, :nt], lhsT=wgu_sb[:, k, ts(mp, 128)],
                            rhs=x_sb[:, k, :nt], start=(k == 0), stop=(k == KD - 1))
                    psu = pu.tile([128, NT], dt.float32, tag="pu")
                    for k in range(KD):
                        nc.tensor.matmul(
                            psu[:, :nt], lhsT=wgu_sb[:, k, ds(F + mp * 128, 128)],
                            rhs=x_sb[:, k, :nt], start=(k == 0), stop=(k == KD - 1))
                    sil = spool.tile([128, NT], dt.bfloat16, tag="sil")
                    nc.scalar.activation(sil[:, :nt], psg[:, :nt],
                                         mybir.ActivationFunctionType.Silu)
                    nc.vector.tensor_mul(h_sb[:, mp, :nt], sil[:, :nt], psu[:, :nt])
                for m in range(MD):
                    pso = po.tile([128, NT], dt.float32, tag="po")
                    for k in range(KF):
                        nc.tensor.matmul(
                            pso[:, :nt], lhsT=wd_sb[:, k, ts(m, 128)],
                            rhs=h_sb[:, k, :nt], start=(k == 0), stop=(k == KF - 1))
                    o_sb = opool.tile([128, NT], dt.float32, tag="o")
                    nc.vector.tensor_copy(o_sb[:, :nt], pso[:, :nt])
                    nc.sync.dma_start(outT_r[:, m, n0:n0 + nt], o_sb[:, :nt])

        if hw_loop:
            with tc.For_i(0, hw_loop, 1):
                body()
        else:
            body()
    nc.finalize()
    return nc


def build_nc_wide(C, hw_loop=0):
    """Variant: 1024-token compute chunks with [128,1024] PSUM tiles.

    - halves ACT/DVE eviction instruction count (wide silu/mul)
    - consecutive matmuls share the same lhsT (LDW dedup opportunity)
    - PSUM banks: pg 2x2 + pu 1x2 + po 2x1 = 8
    """
    nc = bacc.Bacc("TRN2", target_bir_lowering=False, debug=False, num_devices=E)
    dt = mybir.dt
    NW = 1024
    xT = nc.dram_tensor("xT", [D, C], dt.bfloat16, kind="ExternalInput")
    wgu = nc.dram_tensor("wguT", [D, F2], dt.bfloat16, kind="ExternalInput")
    wd = nc.dram_tensor("wdT", [F, D], dt.bfloat16, kind="ExternalInput")
    outT = nc.dram_tensor("outT", [D, C], dt.float32, kind="ExternalOutput")

    with TileContext(nc) as tc, ExitStack() as ctx:
        wpool = ctx.enter_context(tc.tile_pool(name="weights", bufs=1))
        wgu_sb = wpool.tile([128, KD, F2], dt.bfloat16, tag="wgu")
        nc.sync.dma_start(wgu_sb[:], wgu.rearrange("(k p) f -> p k f", p=128))
        wd_sb = wpool.tile([128, KF, D], dt.bfloat16, tag="wd")
        nc.sync.dma_start(wd_sb[:], wd.rearrange("(k p) f -> p k f", p=128))

        xpool = ctx.enter_context(tc.tile_pool(name="x", bufs=1))
        hpool = ctx.enter_context(tc.tile_pool(name="h", bufs=3))
        spool = ctx.enter_context(tc.tile_pool(name="silu", bufs=3))
        opool = ctx.enter_context(tc.tile_pool(name="o", bufs=4))
        pg = ctx.enter_context(tc.tile_pool(name="pg", bufs=2, space="PSUM"))
        pu = ctx.enter_context(tc.tile_pool(name="pu", bufs=1, space="PSUM"))
        po = ctx.enter_context(tc.tile_pool(name="po", bufs=2, space="PSUM"))

        xT_r = xT.rearrange("(k p) t -> p k t", p=128)
        outT_r = outT.rearrange("(m p) t -> p m t", p=128)

        def do_chunk(n0, nw):
            # nw tokens starting at n0; nw in {1024, C % 1024}
            nh = (nw + NT - 1) // NT  # h sub-chunks of <=512
            x_sb = xpool.tile([128, KD, NW], dt.bfloat16, tag="x")
            nc.sync.dma_start(x_sb[:, :, :nw], xT_r[:, :, n0:n0 + nw])
            h_sbs = [hpool.tile([128, KF, NT], dt.bfloat16, tag="h",
                                name=f"h_{n0}_{s}")
                     for s in range(nh)]
            for mp in range(KF):
                psg = pg.tile([128, NW], dt.float32, tag="pg")
                for k in range(KD):
                    for s in range(nh):
                        w = min(NT, nw - s * NT)
                        nc.tensor.matmul(
                            psg[:, s * NT:s * NT + w],
                            lhsT=wgu_sb[:, k, ts(mp, 128)],
                            rhs=x_sb[:, k, s * NT:s * NT + w],
                            start=(k == 0), stop=(k == KD - 1))
                psu = pu.tile([128, NW], dt.float32, tag="pu")
                for k in range(KD):
                    for s in range(nh):
                        w = min(NT, nw - s * NT)
                        nc.tensor.matmul(
                            psu[:, s * NT:s * NT + w],
                            lhsT=wgu_sb[:, k, ds(F + mp * 128, 128)],
                            rhs=x_sb[:, k, s * NT:s * NT + w],
                            start=(k == 0), stop=(k == KD - 1))
                sil = spool.tile([128, NW], dt.bfloat16, tag="sil")
                nc.scalar.activation(sil[:, :nw], psg[:, :nw],
                                     mybir.ActivationFunctionType.Silu)
                for s in range(nh):
                    w = min(NT, nw - s * NT)
                    nc.vector.tensor_mul(h_sbs[s][:, mp, :w],
                                         sil[:, s * NT:s * NT + w],
                                         psu[:, s * NT:s * NT + w])
            for m in range(MD):
                for s in range(nh):
                    w = min(NT, nw - s * NT)
                    pso = po.tile([128, NT], dt.float32, tag="po")
                    for k in range(KF):
                        nc.tensor.matmul(
                            pso[:, :w], lhsT=wd_sb[:, k, ts(m, 128)],
                            rhs=h_sbs[s][:, k, :w],
                            start=(k == 0), stop=(k == KF - 1))
                    o_sb = opool.tile([128, NT], dt.float32, tag="o")
                    nc.vector.tensor_copy(o_sb[:, :w], pso[:, :w])
                    nc.sync.dma_start(outT_r[:, m, n0 + s * NT:n0 + s * NT + w],
                                      o_sb[:, :w])

        def body():
            for n0 in range(0, C, NW):
                do_chunk(n0, min(NW, C - n0))

        if hw_loop:
            with tc.For_i(0, hw_loop, 1):
                body()
        else:
            body()
    nc.finalize()
    return nc


def get_nc(C):
    # build_nc_v3: consumption-ordered weight streaming + double-buffered x.
    # TimelineSim: 342 us vs 365 us for build_nc_big (PE busy 337.5 us is the
    # bf16 streaming floor; v3 removes most cold-start DMA stall).
    if C not in _nc_cache:
        _nc_cache[C] = build_nc_v3(C)
    return _nc_cache[C]


def build_nc_v3(C, hw_loop=0, preload_weights=False, repeats=1,
                out_f16=True, even_slices=False):
    """1536-token chunks like build_nc_big, plus:

    - weight DMAs split [128, 512] per (k, f-block) and emitted in
      consumption order (gate f-blocks k-major, then up, then down),
      interleaved BEHIND the first chunk's x tiles, so the first matmul
      group unblocks after ~0.5 MB and the DMA stream stays ahead of the
      PE's weight-consumption order throughout.
    - xpool bufs=2: chunk n+1's x streams during chunk n's compute.
    - preload_weights: emit weight DMAs outside the hw_loop body (timing
      builds only) so a looped run models the single-shot steady state
      instead of re-streaming 17 MB of weights every rep.
    """
    nc = bacc.Bacc("TRN2", target_bir_lowering=False, debug=False, num_devices=E)
    dt = mybir.dt
    NB = 1536
    FB = 512  # weight f-block DMA granularity
    xT = nc.dram_tensor("xT", [D, C], dt.float16, kind="ExternalInput")
    wgu = nc.dram_tensor("wguT", [D, F2], dt.float16, kind="ExternalInput")
    wd = nc.dram_tensor("wdT", [F, D], dt.float16, kind="ExternalInput")
    out_dt = dt.float16 if out_f16 else dt.float32
    outT = nc.dram_tensor("outT", [D, C], out_dt, kind="ExternalOutput")

    with TileContext(nc) as tc, ExitStack() as ctx:
        wpool = ctx.enter_context(tc.tile_pool(name="weights", bufs=1))
        wgu_k = [wpool.tile([128, F2], dt.float16, tag=f"wgu{k}",
                            name=f"wgu{k}") for k in range(KD)]
        wd_k = [wpool.tile([128, D], dt.float16, tag=f"wd{k}",
                           name=f"wd{k}") for k in range(KF)]

        xpool = ctx.enter_context(tc.tile_pool(name="x", bufs=2))
        ghpool = ctx.enter_context(tc.tile_pool(name="gh", bufs=1))
        opool = ctx.enter_context(tc.tile_pool(name="o", bufs=4))
        pp = ctx.enter_context(tc.tile_pool(name="pp", bufs=2, space="PSUM"))
        po = ctx.enter_context(tc.tile_pool(name="po", bufs=2, space="PSUM"))

        xT_r = xT.rearrange("(k p) t -> p k t", p=128)
        outT_r = outT.rearrange("(m p) t -> p m t", p=128)

        def emit_weight_dmas(first_x=None):
            # gate f-blocks (cols 0..F), k-major inside each block, then up
            # (cols F..2F), then down weights; the first chunk's x k-tiles
            # lead the stream so the first gate k-chain isn't x-starved.
            if first_x is not None:
                # interleave G0's k-tiles with x's k-tiles, weight first:
                # LDW k only waits on its 128 KB weight block while the
                # matmul additionally waits on the 145 KB x k-tile.
                # (512-col blocks: 1 KB contiguous lines — under 512 B the
                # DMA engines pay a 2x latency multiplier.)
                x_sb, n0, nw = first_x
                for k in range(KD):
                    nc.sync.dma_start(wgu_k[k][:, 0:FB],
                                      wgu[k * 128:(k + 1) * 128, 0:FB])
                    nc.sync.dma_start(x_sb[:, k, :nw], xT_r[:, k, n0:n0 + nw])
                fb_start = FB
            else:
                fb_start = 0
            for fb in range(fb_start, F2, FB):
                for k in range(KD):
                    nc.sync.dma_start(wgu_k[k][:, fb:fb + FB],
                                      wgu[k * 128:(k + 1) * 128, fb:fb + FB])
            for k in range(KF):
                nc.sync.dma_start(wd_k[k][:], wd[k * 128:(k + 1) * 128, :])

        def slices(nw):
            # DEFAULT (even_slices=False): 512-aligned sub-chunks + tail.
            # Sub-chunk starts MUST be multiples of 512 — each matmul chain
            # accumulates into ps[:, s:s+w], and a chain that starts mid-bank
            # spans a PSUM bank boundary (512 fp32/bank), which Bass does not
            # reject and which silently corrupts accumulation on HW (NaNs).
            # even_slices=True (288/280-style widths) hits exactly that and
            # is kept only as a record of the failed experiment.
            if not even_slices:
                return [(s, min(NT, nw - s)) for s in range(0, nw, NT)]
            ns = -(-nw // NT)
            base = -(-nw // (ns * 8)) * 8
            out, pos = [], 0
            while pos < nw:
                w = min(base, nw - pos)
                out.append((pos, w))
                pos += w
            return out

        def do_chunk(n0, nw, x_pre=None, is_last=False):
            if x_pre is None:
                x_sb = xpool.tile([128, KD, NB], dt.float16, tag="x")
                for k in range(KD):
                    nc.sync.dma_start(x_sb[:, k, :nw], xT_r[:, k, n0:n0 + nw])
            else:
                x_sb = x_pre
            gh = ghpool.tile([128, KF, NB], dt.float16, tag="gh")
            for phase in (0, 1):  # 0: gate+silu, 1: up+mul-in-place
                for mp in range(KF):
                    ps = pp.tile([128, NB], dt.float32, tag="pp",
                                 name=f"ps_{n0}_{phase}_{mp}")
                    f0 = mp * 128 if phase == 0 else F + mp * 128
                    for k in range(KD):
                        for s, w in slices(nw):
                            nc.tensor.matmul(
                                ps[:, s:s + w],
                                lhsT=wgu_k[k][:, ds(f0, 128)],
                                rhs=x_sb[:, k, s:s + w],
                                start=(k == 0), stop=(k == KD - 1))
                    if phase == 0:
                        nc.scalar.activation(gh[:, mp, :nw], ps[:, :nw],
                                             mybir.ActivationFunctionType.Silu)
                    else:
                        nc.vector.tensor_mul(gh[:, mp, :nw], gh[:, mp, :nw],
                                             ps[:, :nw])
            for m in range(MD):
                for s, w in slices(nw):
                    final = is_last and m == MD - 1 and s + w >= nw
                    # final psum group: two half-width chains so the last
                    # copy+DMA pipelines behind the second chain's matmuls
                    parts = [(s, w - w // 2), (s + w - w // 2, w // 2)] \
                        if final and w > 128 else [(s, w)]
                    for sp, wp in parts:
                        pso = po.tile([128, NT], dt.float32, tag="po",
                                      name=f"pso_{n0}_{m}_{sp}")
                        for k in range(KF):
                            nc.tensor.matmul(
                                pso[:, :wp], lhsT=wd_k[k][:, ts(m, 128)],
                                rhs=gh[:, k, sp:sp + wp],
                                start=(k == 0), stop=(k == KF - 1))
                        o_sb = opool.tile([128, NT], out_dt, tag="o",
                                          name=f"o_{n0}_{m}_{sp}")
                        nc.vector.tensor_copy(o_sb[:, :wp], pso[:, :wp])
                        nc.sync.dma_start(outT_r[:, m, n0 + sp:n0 + sp + wp],
                                          o_sb[:, :wp])

        # smallest chunk first: the cold-start stall is gated on the first
        # chunk's x DMA, so lead with the cheapest one — but the first chunk
        # must stay big enough that its gate+up compute covers the 8.4 MB
        # gate/up weight stream (>= ~400 tokens; sim-validated optimum).
        chunks = [(n0, min(NB, C - n0)) for n0 in range(0, C, NB)]
        chunks.sort(key=lambda c: c[1])

        def body(first):
            for i, (n0, nw) in enumerate(chunks):
                last = i == len(chunks) - 1
                if i == 0 and first:
                    x_sb = xpool.tile([128, KD, NB], dt.float16, tag="x")
                    emit_weight_dmas(first_x=(x_sb, n0, nw))
                    do_chunk(n0, nw, x_pre=x_sb, is_last=last)
                else:
                    do_chunk(n0, nw, is_last=last)

        if hw_loop:
            if preload_weights:
                emit_weight_dmas()
            with tc.For_i(0, hw_loop, 1):
                body(first=not preload_weights)
        elif repeats > 1:
            # unrolled steady-state body (TimelineSim can't branch): weights
            # once, then the x+compute+out body repeated
            emit_weight_dmas()
            for _ in range(repeats):
                body(first=False)
        else:
            body(first=True)
    nc.finalize()
    return nc


def build_nc_big(C, hw_loop=0):
    """Variant: 1536-token chunks ([128,1536] 3-bank PSUM tiles).

    Streams 3x512 tokens per weight load (LDW count 1920 -> ~768), evicts
    gate via silu into a chunk-resident SBUF tensor, then multiplies the up
    projection into it in place. PSUM: pp 2x3 + po 2x1 = 8 banks.
    """
    nc = bacc.Bacc("TRN2", target_bir_lowering=False, debug=False, num_devices=E)
    dt = mybir.dt
    NB = 1536
    xT = nc.dram_tensor("xT", [D, C], dt.float16, kind="ExternalInput")
    wgu = nc.dram_tensor("wguT", [D, F2], dt.float16, kind="ExternalInput")
    wd = nc.dram_tensor("wdT", [F, D], dt.float16, kind="ExternalInput")
    outT = nc.dram_tensor("outT", [D, C], dt.float32, kind="ExternalOutput")

    with TileContext(nc) as tc, ExitStack() as ctx:
        # per-k weight tiles with separate DMAs; the first chunk's x tiles
        # are DMA'd BEFORE the weights (see do_chunk) so the PE's first
        # matmul group is gated on ~4 MB, not the full 16 MB input set.
        wpool = ctx.enter_context(tc.tile_pool(name="weights", bufs=1))
        wgu_k = [wpool.tile([128, F2], dt.float16, tag=f"wgu{k}",
                            name=f"wgu{k}") for k in range(KD)]
        wd_k = [wpool.tile([128, D], dt.float16, tag=f"wd{k}",
                           name=f"wd{k}") for k in range(KF)]

        xpool = ctx.enter_context(tc.tile_pool(name="x", bufs=1))
        ghpool = ctx.enter_context(tc.tile_pool(name="gh", bufs=1))
        opool = ctx.enter_context(tc.tile_pool(name="o", bufs=4))
        pp = ctx.enter_context(tc.tile_pool(name="pp", bufs=2, space="PSUM"))
        po = ctx.enter_context(tc.tile_pool(name="po", bufs=2, space="PSUM"))

        xT_r = xT.rearrange("(k p) t -> p k t", p=128)
        outT_r = outT.rearrange("(m p) t -> p m t", p=128)

        def slices(nw):
            return [(s, min(NT, nw - s)) for s in range(0, nw, NT)]

        def do_chunk(n0, nw):
            x_sb = xpool.tile([128, KD, NB], dt.float16, tag="x")
            if first[0]:
                first[0] = False
                # interleave x and gate/up weight k-tiles so the first
                # matmul group's operands stream in consumption order
                for k in range(KD):
                    nc.sync.dma_start(x_sb[:, k, :nw], xT_r[:, k, n0:n0 + nw])
                    # first 512 f-columns land first so the k-th LDW of the
                    # first gate group unblocks after ~0.4 MB, not 1 MB
                    nc.sync.dma_start(wgu_k[k][:, :NT],
                                      wgu[k * 128:(k + 1) * 128, :NT])
                    nc.sync.dma_start(wgu_k[k][:, NT:],
                                      wgu[k * 128:(k + 1) * 128, NT:])
                for k in range(KF):
                    nc.sync.dma_start(wd_k[k][:], wd[k * 128:(k + 1) * 128, :])
            else:
                for k in range(KD):
                    nc.sync.dma_start(x_sb[:, k, :nw], xT_r[:, k, n0:n0 + nw])
            gh = ghpool.tile([128, KF, NB], dt.float16, tag="gh")
            for phase in (0, 1):  # 0: gate+silu, 1: up+mul-in-place
                for mp in range(KF):
                    ps = pp.tile([128, NB], dt.float32, tag="pp",
                                 name=f"ps_{n0}_{phase}_{mp}")
                    f0 = mp * 128 if phase == 0 else F + mp * 128
                    for k in range(KD):
                        for s, w in slices(nw):
                            nc.tensor.matmul(
                                ps[:, s:s + w],
                                lhsT=wgu_k[k][:, ds(f0, 128)],
                                rhs=x_sb[:, k, s:s + w],
                                start=(k == 0), stop=(k == KD - 1))
                    if phase == 0:
                        nc.scalar.activation(gh[:, mp, :nw], ps[:, :nw],
                                             mybir.ActivationFunctionType.Silu)
                    else:
                        nc.vector.tensor_mul(gh[:, mp, :nw], gh[:, mp, :nw],
                                             ps[:, :nw])
            for m in range(MD):
                for s, w in slices(nw):
                    pso = po.tile([128, NT], dt.float32, tag="po",
                                  name=f"pso_{n0}_{m}_{s}")
                    for k in range(KF):
                        nc.tensor.matmul(
                            pso[:, :w], lhsT=wd_k[k][:, ts(m, 128)],
                            rhs=gh[:, k, s:s + w],
                            start=(k == 0), stop=(k == KF - 1))
                    o_sb = opool.tile([128, NT], dt.float32, tag="o",
                                      name=f"o_{n0}_{m}_{s}")
                    nc.vector.tensor_copy(o_sb[:, :w], pso[:, :w])
                    nc.sync.dma_start(outT_r[:, m, n0 + s:n0 + s + w],
                                      o_sb[:, :w])

        first = [True]

        def body():
            # smallest chunk first: the cold-start stall is gated on the
            # first chunk's x DMA, so lead with the cheapest one
            chunks = [(n0, min(NB, C - n0)) for n0 in range(0, C, NB)]
            chunks.sort(key=lambda c: c[1])
            for n0, nw in chunks:
                do_chunk(n0, nw)



        if hw_loop:
            with tc.For_i(0, hw_loop, 1):
                body()
        else:
            body()
    nc.finalize()
    return nc


def route(x, expert_indices):
    """Sort tokens by expert; return (order, counts, capacity C)."""
    idx = np.asarray(expert_indices)
    order = np.argsort(idx, kind="stable")
    counts = np.bincount(idx, minlength=E).astype(np.int64)
    C = max(NT, int(-(-counts.max() // 8) * 8))
    return order, counts, C


def make_in_maps(x, expert_indices, gate_up_weight, down_weight):
    order, counts, C = route(x, expert_indices)
    x_sorted = np.asarray(x, dtype=np.float32)[order]
    offs = np.zeros(E + 1, dtype=np.int64)
    np.cumsum(counts, out=offs[1:])
    wguT = np.ascontiguousarray(
        np.transpose(np.asarray(gate_up_weight), (0, 2, 1))).astype(F16)
    wdT = np.ascontiguousarray(
        np.transpose(np.asarray(down_weight), (0, 2, 1))).astype(F16)
    in_maps = []
    for e in range(E):
        xe = np.zeros((C, D), dtype=np.float32)
        xe[: counts[e]] = x_sorted[offs[e]: offs[e + 1]]
        in_maps.append({
            "xT": np.ascontiguousarray(xe.T).astype(F16),
            "wguT": wguT[e],
            "wdT": wdT[e],
        })
    return in_maps, order, counts, C


def assemble_output(results, order, counts):
    T = int(counts.sum())
    out = np.empty((T, D), dtype=np.float32)
    offs = np.zeros(E + 1, dtype=np.int64)
    np.cumsum(counts, out=offs[1:])
    sorted_out = np.empty((T, D), dtype=np.float32)
    for e in range(E):
        sorted_out[offs[e]: offs[e + 1]] = results[e]["outT"].T[: counts[e]]
    out[order] = sorted_out
    return out


def kernel(x, expert_indices, gate_up_weight, down_weight):
    in_maps, order, counts, C = make_in_maps(
        x, expert_indices, gate_up_weight, down_weight)
    nc = get_nc(C)
    res = run_bass_kernel_spmd(nc, in_maps, core_ids=list(range(E)))
    return assemble_output(res.results, order, counts)

